# revision 29
# baseline (speedup 1.0000x reference)
"""Multi-headed attention Trainium2 kernel (v2: fused projection/attention pipeline).

Problem: B=4, S=2048, D=1024, H=16, dk=dv=64, fp32.
  q = einsum("bsd,hdk->bhsk", x, W_Q); k,v similar
  scores = q@k.T/8; attn = softmax(scores); out = attn@v
  y = concat_heads(out) @ W_O

Sharding: 8 cores = 4-way data parallel (batch) x 2-way tensor parallel
(head groups of 8). Core c handles batch c%4, heads 8*(c//4)..+8. Each core
returns a partial y for its batch; host sums the two head-group partials.

Per-core dataflow (S=2048, D=1024, HL=8 local heads):
  - x is cast to bf16 on DVE and transposed by the DMA xbar into one
    resident xT tile [128, ND*S] (no PE/PSUM involvement). x loads go out
    on the ACT HWDGE ring, everything else on the SP ring, so the lead-in
    is not serialized on one DMA ring.
  - K is projected pair-packed (bf16 operands, fp32 accum) into resident
    f32r kt tiles; Q is projected on the fly per (s-chunk, pair) inside
    the attention loop (no DRAM spill).
  - V is projected in two 4-head halves into a resident bf16 [t, v|1]
    layout with a ones column per head (softmax denominator comes out of
    the AV matmul's row 64).
  - attention per (s-chunk, pair): scores^T [t, s] = kt.T @ qT via K=64
    row-packed f32r matmuls, exp on ACT straight from PSUM with fused 1/8
    scale (no max subtraction: |scores| < ~12 so fp32 exp is exact),
    bf16 exp tiles, AV+denominator matmul into PSUM, immediately copied
    to SBUF (frees the accumulator bank), normalize via reciprocal +
    partition_broadcast + mul, bf16 output projection per s-chunk.
  - K/V projection units that phase B does not need yet are emitted
    interleaved into the attention loop (background queue, half-unit
    granularity) so PE slack under the ACT-bound exp stream absorbs them
    instead of delaying the first exp.

Measured end-to-end rel err ~5e-3 (gate 2e-2).
"""

import numpy as np
import ml_dtypes

import concourse.bacc as bacc
import concourse.bass as bass
import concourse.mybir as mybir
import concourse.tile as tile

F32 = mybir.dt.float32
F32R = mybir.dt.float32r
BF16 = mybir.dt.bfloat16
P = 128
DK = 64  # per-head dim; also dv
VW = DK + 1  # v columns + ones column


def build_nc(S, D, HL, num_devices=8, n_iters=1, cfg=None):
    """Build the per-core Bass program. S seq len, D model dim, HL local heads."""
    NSC = S // 512  # s-chunks
    NT = S // P  # t-tiles
    ND = D // P  # d-tiles
    NPAIR = HL // 2
    NDC = max(1, D // 512)  # output d chunks
    DC = min(D, 512)
    HH = HL // 2  # heads per V half
    scale = 1.0 / np.sqrt(np.float64(DK))
    cfg = dict(cfg or {})
    psS_bufs = cfg.get("psS_bufs", 2)
    psB_bufs = cfg.get("psB_bufs", 4)
    et_bufs = cfg.get("et_bufs", 6)
    ot_bufs = cfg.get("ot_bufs", 8)
    po_bufs = cfg.get("po_bufs", 4)
    interleave = cfg.get("interleave", True)
    drain_per_g = cfg.get("drain_per_g", 1)

    nc = bacc.Bacc("TRN2", target_bir_lowering=False, debug=False,
                   num_devices=num_devices)

    x = nc.dram_tensor("x", [S, D], F32, kind="ExternalInput").ap()
    # host-preshuffled weights, partition-major with contiguous rows (bf16):
    # wq/wk [NPAIR, 128, ND*128]: row p = [w(2p)[dt*128+p, :] | w(2p+1)[dt*128+p, :]]_dt
    # wv [128, ND*HL*64]: row p = [wv[hl][dt*128+p, :]]_(dt, hl)
    # wo [128, NPAIR*D]: row p = [wo[pr*128+p, :]]_pr
    wq = nc.dram_tensor("wq", [NPAIR, P, ND * P], BF16, kind="ExternalInput").ap()
    wk = nc.dram_tensor("wk", [NPAIR, P, ND * P], BF16, kind="ExternalInput").ap()
    wv = nc.dram_tensor("wv", [P, ND * HL * DK], BF16, kind="ExternalInput").ap()
    wo = nc.dram_tensor("wo", [P, NPAIR * D], BF16, kind="ExternalInput").ap()
    y = nc.dram_tensor("y", [S, D], F32, kind="ExternalOutput").ap()

    from contextlib import ExitStack

    with tile.TileContext(nc) as tc:
        with ExitStack() as ctx:
            pool = lambda name, bufs, **kw: ctx.enter_context(
                tc.tile_pool(name=name, bufs=bufs, **kw)
            )
            persist = pool("persist", 1)
            xl_p = pool("xload", 4)
            xc_p = pool("xcast", 4)
            wqk_p = pool("wqk", 3)
            qtb_p = pool("qtb", 2)
            et_p = pool("exp", et_bufs)
            ot_p = pool("ot", ot_bufs)
            po_p = pool("po", po_bufs)
            y_p = pool("ysb", 3)
            rl_p = pool("rl", 2)
            rb_p = pool("rb", 2)
            tmp_p = pool("tmp", 2)
            rl0_p = pool("rl0", 2)
            psS = pool("psS", psS_bufs, space="PSUM")
            psB = pool("psB", psB_bufs, space="PSUM")
            if n_iters > 1:
                ctx.enter_context(tc.For_i(0, n_iters, 1))

            # --- persistent tiles ---
            # xT: one tile [128, ND*S] bf16; d-tile dt at cols [dt*S, (dt+1)*S)
            xtall = persist.tile([P, ND * S], BF16, tag="xtall")
            xt = [xtall[:, d * S : (d + 1) * S] for d in range(ND)]
            xt_3d = xtall[:].rearrange("p (t s) -> p t s", t=ND)
            kt = [persist.tile([P, S], F32R, tag=f"kt{p}", name=f"kt{p}")
                  for p in range(NPAIR)]
            v_ones = persist.tile([P, NT * HL * VW], BF16, tag="vones")
            wos_all = persist.tile([P, NPAIR * D], BF16, tag="wo")
            wv_sb = persist.tile([P, ND * HL * DK], BF16, tag="wv")
            wkt = [persist.tile([P, ND * P], BF16, tag=f"wk{p}", name=f"wk{p}")
                   for p in range(NPAIR)]
            wqt = [persist.tile([P, ND * P], BF16, tag=f"wq{p}", name=f"wq{p}")
                   for p in range(NPAIR)]

            # --- lead-in ---
            # wk first: it heads the SP ring FIFO so K proj isn't stuck
            # behind the transpose stream
            for p in range(NPAIR):
                nc.sync.dma_start(wkt[p][:], wk[p])

            # x loads (ACT ring first half / SP ring second half) + cast +
            # xbar transpose, so neither DMA ring serializes the x path
            for sc in range(NSC):
                for st in range(4):
                    row = sc * 4 + st
                    xl = xl_p.tile([P, D], F32, tag="xl")
                    # first half of x on the ACT ring (exp stream hasn't
                    # started), second half on the SP ring
                    (nc.scalar if row < 8 else nc.sync).dma_start(
                        xl[:], x[row * P : (row + 1) * P, :]
                    )
                    xc = xc_p.tile([P, D], BF16, tag="xc")
                    nc.vector.tensor_copy(xc[:], xl[:])
                    # out[d % 128, dt, s] = xc[s, dt*128 + d%128]
                    nc.sync.dma_start_transpose(
                        xt_3d[:, :, row * P : (row + 1) * P], xc[:]
                    )

            # --- emission units (half-unit granularity for drain pacing) ---
            pending = {}

            def emit_kproj(p, sc, phase):
                """Project K pair p, s-chunk sc; phase 0/1 = dt 0-3 / 4-7."""
                if phase == 0:
                    ps = psB.tile([P, 512], F32, tag="ps", name=f"kp{p}_{sc}")
                    pending[("k", p, sc)] = ps
                else:
                    ps = pending.pop(("k", p, sc))
                for dt in range(phase * 4, phase * 4 + 4):
                    nc.tensor.matmul(
                        ps[:], wkt[p][:, dt * P : (dt + 1) * P],
                        xt[dt][:, sc * 512 : (sc + 1) * 512],
                        start=(dt == 0), stop=(dt == ND - 1),
                    )
                if phase == 1:
                    nc.vector.tensor_copy(kt[p][:, sc * 512 : (sc + 1) * 512], ps[:])

            def emit_vproj(half, tt, phase):
                """Project V half (4 heads), t-tile tt; phase 0/1 = dt 0-3 / 4-7."""
                if phase == 0:
                    ps = psB.tile([P, 512], F32, tag="ps", name=f"vp{half}_{tt}")
                    pending[("v", half, tt)] = ps
                else:
                    ps = pending.pop(("v", half, tt))
                for dt in range(phase * 4, phase * 4 + 4):
                    nc.tensor.matmul(
                        ps[:, : HH * DK],
                        xt[dt][:, tt * P : (tt + 1) * P],
                        wv_sb[:, (dt * HL + half * HH) * DK : (dt * HL + (half + 1) * HH) * DK],
                        start=(dt == 0), stop=(dt == ND - 1),
                    )
                if phase == 1:
                    nc.vector.tensor_copy(
                        v_ones[:].rearrange("p (t h c) -> p t h c", h=HL, c=VW)[
                            :, tt, half * HH : (half + 1) * HH, :DK
                        ],
                        ps[:, : HH * DK].rearrange("p (h k) -> p h k", h=HH),
                    )

            def emit_wo(ots, sc, dc, st):
                """Output projection chunk: y[sc-rows st, dc] from 4 pair ots."""
                psy = psB.tile([P, 512], F32, tag="ps")
                for p in range(NPAIR):
                    nc.tensor.matmul(
                        psy[:, :DC],
                        ots[p][:, st * P : (st + 1) * P],
                        wos_all[:, p * D + dc * DC : p * D + (dc + 1) * DC],
                        start=(p == 0), stop=(p == NPAIR - 1),
                    )
                ys = y_p.tile([P, DC], F32, tag="ysb")
                nc.vector.tensor_copy(ys[:], psy[:, :DC])
                nc.sync.dma_start(
                    y[(sc * 4 + st) * P : (sc * 4 + st + 1) * P,
                      dc * DC : (dc + 1) * DC],
                    ys[:],
                )

            bg = []

            def run_unit(u):
                if u[0] == "k":
                    emit_kproj(u[1], u[2], u[3])
                elif u[0] == "v":
                    emit_vproj(u[1], u[2], u[3])
                else:
                    emit_wo(u[1], u[2], u[3], u[4])

            def xdep(u):
                """Highest x-chunk a unit depends on (-1 = none)."""
                if u[0] == "k":
                    return u[2]
                if u[0] == "v":
                    return u[2] // 4
                return -1

            def drain(n, max_x=99):
                """Emit up to n background units whose x-chunk is ready."""
                for _ in range(n):
                    if bg and xdep(bg[0]) <= max_x:
                        run_unit(bg.pop(0))

            def need(u):
                while u in bg:
                    run_unit(bg.pop(0))

            emit_kproj(0, 0, 0)
            emit_kproj(0, 0, 1)

            # ones column (col DK of each per-head block) + remaining weights
            nc.vector.memset(
                v_ones[:].rearrange("p (t h c) -> p (t h) c", h=HL, c=VW)[
                    :, :, DK : DK + 1
                ],
                1.0,
            )
            nc.scalar.dma_start(wv_sb[:], wv[:])

            # consumption order: pair 0's g-loop pulls k(0, chunk) and
            # v(0, tile) incrementally; later pairs' K chunks follow.
            units = []
            for sc in range(1, NSC):  # k(0,sc) just-in-time with v(0) tiles
                units += [("v", 0, tt, ph) for tt in range((sc - 1) * 4, sc * 4)
                          for ph in range(2)]
                units += [("k", 0, sc, ph) for ph in range(2)]
            units += [("v", 0, tt, ph) for tt in range(12, 16) for ph in range(2)]
            units += [("k", 1, sc, ph) for sc in range(NSC) for ph in range(2)]
            for sc in range(NSC):  # k(2,sc) just-in-time with v(1) tiles
                units += [("k", 2, sc, ph) for ph in range(2)]
                units += [("v", 1, tt, ph) for tt in range(sc * 4, (sc + 1) * 4)
                          for ph in range(2)]
            units += [("k", 3, sc, ph) for sc in range(NSC) for ph in range(2)]
            if interleave:
                bg.extend(units)
            else:
                for u in units:
                    run_unit(u)

            nc.scalar.dma_start(wos_all[:], wo[:])

            # --- attention + output projection ---
            for sc in range(NSC):
                ots = []
                for p in range(NPAIR):
                    # Q projection for (p, sc)
                    wt = wqk_p.tile([P, ND * P], BF16, tag="wqk")
                    nc.sync.dma_start(wt[:], wq[p])
                    psq = psB.tile([P, 512], F32, tag="ps")
                    for dt in range(ND):
                        nc.tensor.matmul(
                            psq[:], wt[:, dt * P : (dt + 1) * P],
                            xt[dt][:, sc * 512 : (sc + 1) * 512],
                            start=(dt == 0), stop=(dt == ND - 1),
                        )
                    qtb = qtb_p.tile([P, 512], F32R, tag="qtb")
                    nc.vector.tensor_copy(qtb[:], psq[:])

                    po_e = psB.tile([P, 512], F32, tag="ps")
                    po_o = psB.tile([P, 512], F32, tag="ps")
                    half = 2 * p // HH
                    for g in range(NT):
                        need(("k", p, g // 4, 1))
                        need(("v", half, g, 1))
                        pse = psS.tile([P, 1024], F32, tag="sc")
                        for h in range(2):
                            nc.tensor.matmul(
                                pse[:, h * 512 : (h + 1) * 512],
                                kt[p][h * DK : (h + 1) * DK, g * P : (g + 1) * P],
                                qtb[h * DK : (h + 1) * DK, :],
                                start=True, stop=True,
                            )
                        et = et_p.tile([P, 1024], BF16, tag="exp")
                        nc.scalar.activation(
                            et[:], pse[:], mybir.ActivationFunctionType.Exp,
                            scale=float(scale),
                        )
                        for h, po in ((0, po_e), (1, po_o)):
                            nc.tensor.matmul(
                                po[:VW, :],
                                v_ones[
                                    :,
                                    (g * HL + 2 * p + h) * VW : (g * HL + 2 * p + h + 1) * VW,
                                ],
                                et[:, h * 512 : (h + 1) * 512],
                                start=(g == 0), stop=(g == NT - 1),
                            )
                        # during the first pair, only drain units whose x
                        # chunk has landed (x streams in behind the g-loop)
                        drain(drain_per_g,
                              max_x=(g // 4 + 1) if (sc == 0 and p == 0) else 99)

                    # move accumulators to SBUF (frees PSUM banks), then
                    # normalize: rows 0:64 divided by row 64 (sum of exp)
                    ot = ot_p.tile([P, 512], BF16, tag="ot")
                    ots.append(ot)
                    for h, po in ((0, po_e), (1, po_o)):
                        pos = po_p.tile([VW, 512], F32, tag="po")
                        nc.vector.tensor_copy(pos[:], po[:VW, :])
                        rl = rl_p.tile([VW, 512], F32, tag="rl")
                        nc.vector.reciprocal(rl[DK : DK + 1, :], pos[DK : DK + 1, :])
                        # partition_broadcast reads physical partition 0:
                        # hop the row down first
                        rl0 = rl0_p.tile([1, 512], F32, tag="rl0")
                        nc.sync.dma_start(rl0[:], rl[DK : DK + 1, :])
                        rb = rb_p.tile([DK, 512], F32, tag="rb")
                        nc.gpsimd.partition_broadcast(rb[:], rl0[:], channels=DK)
                        if h == 0:
                            nc.vector.tensor_mul(ot[:DK, :], pos[:DK, :], rb[:])
                        else:
                            tmp = tmp_p.tile([DK, 512], BF16, tag="tmp")
                            nc.vector.tensor_mul(tmp[:], pos[:DK, :], rb[:])
                            nc.sync.dma_start(ot[DK:P, :], tmp[:])

                # output projection for this s-chunk: deferred into the next
                # s-chunk's PE slack so the normalize tail of the last pair
                # never stalls the pipeline
                for dc in range(NDC):
                    for st in range(4):
                        bg.append(("wo", ots, sc, dc, st))
            drain(len(bg))

    nc.compile()
    return nc


_NC_CACHE = {}


def _get_nc(S, D, HL):
    key = (S, D, HL)
    if key not in _NC_CACHE:
        _NC_CACHE[key] = build_nc(S, D, HL)
    return _NC_CACHE[key]


def prep_core_inputs(x_b, wq_l, wk_l, wv_l, wo_l):
    """Per-core input dict from logical per-core slices.

    x_b [S,D]; wq_l/wk_l/wv_l [HL,D,64]; wo_l [HL*64,D]. Weights are
    reshuffled host-side into partition-major layouts (see build_nc).
    """
    S, D = x_b.shape
    HL = wq_l.shape[0]
    ND, NPAIR, NT = D // P, HL // 2, S // P
    bf = ml_dtypes.bfloat16

    def qk_prep(w):
        return np.ascontiguousarray(
            w.reshape(NPAIR, 2, ND, P, DK).transpose(0, 3, 2, 1, 4)
            .reshape(NPAIR, P, ND * P)
        ).astype(bf)

    return {
        "x": np.ascontiguousarray(x_b),
        "wq": qk_prep(wq_l),
        "wk": qk_prep(wk_l),
        "wv": np.ascontiguousarray(
            wv_l.reshape(HL, ND, P, DK).transpose(2, 1, 0, 3)
            .reshape(P, ND * HL * DK)
        ).astype(bf),
        "wo": np.ascontiguousarray(
            wo_l.reshape(NPAIR, P, D).transpose(1, 0, 2).reshape(P, NPAIR * D)
        ).astype(bf),
    }


def make_in_maps(x, W_Q, W_K, W_V, W_O, n_cores=8):
    """Shard full inputs into per-core in_maps (DP over batch x TP over heads)."""
    B = x.shape[0]
    H = W_Q.shape[0]
    n_groups = n_cores // B
    HL = H // n_groups
    in_maps = []
    for c in range(n_cores):
        b, g = c % B, c // B
        hs = slice(g * HL, (g + 1) * HL)
        in_maps.append(prep_core_inputs(
            x[b], W_Q[hs], W_K[hs], W_V[hs],
            W_O[g * HL * DK : (g + 1) * HL * DK],
        ))
    return in_maps


def kernel(x, W_Q, W_K, W_V, W_O):
    from concourse.bass_utils import run_bass_kernel_spmd

    B, S, D = x.shape
    H = W_Q.shape[0]
    n_cores = 8
    HL = H // (n_cores // B)
    nc = _get_nc(S, D, HL)
    in_maps = make_in_maps(x, W_Q, W_K, W_V, W_O, n_cores)
    res = run_bass_kernel_spmd(nc, in_maps, core_ids=list(range(n_cores)))
    y = np.empty((B, S, D), dtype=np.float32)
    for b in range(B):
        y[b] = res.results[b]["y"]
        for g in range(1, n_cores // B):
            y[b] += res.results[g * B + b]["y"]
    return y


# revision 35
# speedup vs baseline: 1.2977x; 1.2977x over previous
"""Multi-headed attention Trainium2 kernel (v2: fused projection/attention pipeline).

Problem: B=4, S=2048, D=1024, H=16, dk=dv=64, fp32.
  q = einsum("bsd,hdk->bhsk", x, W_Q); k,v similar
  scores = q@k.T/8; attn = softmax(scores); out = attn@v
  y = concat_heads(out) @ W_O

Sharding: 8 cores = 4-way data parallel (batch) x 2-way tensor parallel
(head groups of 8). Core c handles batch c%4, heads 8*(c//4)..+8. Each core
returns a partial y for its batch; host sums the two head-group partials.

Per-core dataflow (S=2048, D=1024, HL=8 local heads):
  - x is cast to bf16 on DVE and transposed by the DMA xbar into one
    resident xT tile [128, ND*S] (no PE/PSUM involvement). x loads go out
    on the ACT HWDGE ring, everything else on the SP ring, so the lead-in
    is not serialized on one DMA ring.
  - K is projected pair-packed (bf16 operands, fp32 accum) into resident
    f32r kt tiles; Q is projected on the fly per (s-chunk, pair) inside
    the attention loop (no DRAM spill).
  - V is projected in two 4-head halves into a resident bf16 [t, v|1]
    layout with a ones column per head (softmax denominator comes out of
    the AV matmul's row 64).
  - attention per (s-chunk, pair): scores^T [t, s] = kt.T @ qT via K=64
    row-packed f32r matmuls, exp on ACT straight from PSUM with fused 1/8
    scale (no max subtraction: |scores| < ~12 so fp32 exp is exact),
    bf16 exp tiles, AV+denominator matmul into PSUM, immediately copied
    to SBUF (frees the accumulator bank), normalize via reciprocal +
    partition_broadcast + mul, bf16 output projection per s-chunk.
  - K/V projection units that phase B does not need yet are emitted
    interleaved into the attention loop (background queue, half-unit
    granularity) so PE slack under the ACT-bound exp stream absorbs them
    instead of delaying the first exp.

Measured end-to-end rel err ~5e-3 (gate 2e-2).
"""

import numpy as np
import ml_dtypes

import concourse.bacc as bacc
import concourse.bass as bass
import concourse.mybir as mybir
import concourse.tile as tile

F32 = mybir.dt.float32
F32R = mybir.dt.float32r
BF16 = mybir.dt.bfloat16
P = 128
DK = 64  # per-head dim; also dv
VW = DK + 1  # v columns + ones column


def build_nc(S, D, HL, num_devices=8, n_iters=1, cfg=None):
    """Build the per-core Bass program. S seq len, D model dim, HL local heads."""
    NSC = S // 512  # s-chunks
    NT = S // P  # t-tiles
    ND = D // P  # d-tiles
    NPAIR = HL // 2
    NDC = max(1, D // 512)  # output d chunks
    DC = min(D, 512)
    HH = HL // 2  # heads per V half
    scale = 1.0 / np.sqrt(np.float64(DK))
    cfg = dict(cfg or {})
    psS_bufs = cfg.get("psS_bufs", 2)
    psB_bufs = cfg.get("psB_bufs", 4)
    et_bufs = cfg.get("et_bufs", 6)
    ot_bufs = cfg.get("ot_bufs", 8)
    po_bufs = cfg.get("po_bufs", 4)
    interleave = cfg.get("interleave", True)
    drain_per_g = cfg.get("drain_per_g", 1)

    nc = bacc.Bacc("TRN2", target_bir_lowering=False, debug=False,
                   num_devices=num_devices)

    x = nc.dram_tensor("x", [S, D], F32, kind="ExternalInput").ap()
    # host-preshuffled weights, partition-major with contiguous rows (bf16):
    # wq/wk [NPAIR, 128, ND*128]: row p = [w(2p)[dt*128+p, :] | w(2p+1)[dt*128+p, :]]_dt
    # wv [128, ND*HL*64]: row p = [wv[hl][dt*128+p, :]]_(dt, hl)
    # wo [128, NPAIR*D]: row p = [wo[pr*128+p, :]]_pr
    wq = nc.dram_tensor("wq", [NPAIR, P, ND * P], BF16, kind="ExternalInput").ap()
    wk = nc.dram_tensor("wk", [NPAIR, P, ND * P], BF16, kind="ExternalInput").ap()
    wv = nc.dram_tensor("wv", [P, ND * HL * DK], BF16, kind="ExternalInput").ap()
    wo = nc.dram_tensor("wo", [P, NPAIR * D], BF16, kind="ExternalInput").ap()
    y = nc.dram_tensor("y", [S, D], F32, kind="ExternalOutput").ap()

    from contextlib import ExitStack

    with tile.TileContext(nc) as tc:
        with ExitStack() as ctx:
            pool = lambda name, bufs, **kw: ctx.enter_context(
                tc.tile_pool(name=name, bufs=bufs, **kw)
            )
            persist = pool("persist", 1)
            xl_p = pool("xload", 4)
            xc_p = pool("xcast", 4)
            qtb_p = pool("qtb", 2)
            et_p = pool("exp", et_bufs)
            ot_p = pool("ot", ot_bufs)
            po_p = pool("po", po_bufs)
            y_p = pool("ysb", 3)
            rl_p = pool("rl", 2)
            rb_p = pool("rb", 2)
            tmp_p = pool("tmp", 2)
            rl0_p = pool("rl0", 2)
            psS = pool("psS", psS_bufs, space="PSUM")
            psB = pool("psB", psB_bufs, space="PSUM")
            if n_iters > 1:
                ctx.enter_context(tc.For_i(0, n_iters, 1))

            # --- persistent tiles ---
            # xT: one tile [128, ND*S] bf16; d-tile dt at cols [dt*S, (dt+1)*S)
            xtall = persist.tile([P, ND * S], BF16, tag="xtall")
            xt = [xtall[:, d * S : (d + 1) * S] for d in range(ND)]
            xt_3d = xtall[:].rearrange("p (t s) -> p t s", t=ND)
            kt = [persist.tile([P, S], F32R, tag=f"kt{p}", name=f"kt{p}")
                  for p in range(NPAIR)]
            v_ones = persist.tile([P, NT * HL * VW], BF16, tag="vones")
            wos_all = persist.tile([P, NPAIR * D], BF16, tag="wo")
            wv_sb = persist.tile([P, ND * HL * DK], BF16, tag="wv")
            wkt = [persist.tile([P, ND * P], BF16, tag=f"wk{p}", name=f"wk{p}")
                   for p in range(NPAIR)]
            wqt = [persist.tile([P, ND * P], BF16, tag=f"wq{p}", name=f"wq{p}")
                   for p in range(NPAIR)]

            # --- lead-in ---
            # wk/wq0 first: they head the SP ring FIFO so K/Q projections
            # aren't stuck behind the transpose stream
            for p in range(NPAIR):
                nc.sync.dma_start(wkt[p][:], wk[p])

            # x loads (ACT ring first half / SP ring second half) + cast +
            # xbar transpose, so neither DMA ring serializes the x path
            for sc in range(NSC):
                for st in range(4):
                    row = sc * 4 + st
                    xl = xl_p.tile([P, D], F32, tag="xl")
                    # first half of x on the ACT ring (exp stream hasn't
                    # started), second half on the SP ring
                    (nc.scalar if row < 8 else nc.sync).dma_start(
                        xl[:], x[row * P : (row + 1) * P, :]
                    )
                    xc = xc_p.tile([P, D], BF16, tag="xc")
                    nc.vector.tensor_copy(xc[:], xl[:])
                    # out[d % 128, dt, s] = xc[s, dt*128 + d%128]
                    nc.sync.dma_start_transpose(
                        xt_3d[:, :, row * P : (row + 1) * P], xc[:]
                    )

            # --- emission units (half-unit granularity for drain pacing) ---
            pending = {}

            def emit_kproj(p, sc, phase):
                """Project K pair p, s-chunk sc; phase 0/1 = dt 0-3 / 4-7."""
                if phase == 0:
                    ps = psB.tile([P, 512], F32, tag="ps", name=f"kp{p}_{sc}")
                    pending[("k", p, sc)] = ps
                else:
                    ps = pending.pop(("k", p, sc))
                for dt in range(phase * 4, phase * 4 + 4):
                    nc.tensor.matmul(
                        ps[:], wkt[p][:, dt * P : (dt + 1) * P],
                        xt[dt][:, sc * 512 : (sc + 1) * 512],
                        start=(dt == 0), stop=(dt == ND - 1),
                    )
                if phase == 1:
                    nc.vector.tensor_copy(kt[p][:, sc * 512 : (sc + 1) * 512], ps[:])

            def emit_vproj(half, tt, phase):
                """Project V half (4 heads), t-tile tt; phase 0/1 = dt 0-3 / 4-7."""
                if phase == 0:
                    ps = psB.tile([P, 512], F32, tag="ps", name=f"vp{half}_{tt}")
                    pending[("v", half, tt)] = ps
                else:
                    ps = pending.pop(("v", half, tt))
                for dt in range(phase * 4, phase * 4 + 4):
                    nc.tensor.matmul(
                        ps[:, : HH * DK],
                        xt[dt][:, tt * P : (tt + 1) * P],
                        wv_sb[:, (dt * HL + half * HH) * DK : (dt * HL + (half + 1) * HH) * DK],
                        start=(dt == 0), stop=(dt == ND - 1),
                    )
                if phase == 1:
                    nc.vector.tensor_copy(
                        v_ones[:].rearrange("p (t h c) -> p t h c", h=HL, c=VW)[
                            :, tt, half * HH : (half + 1) * HH, :DK
                        ],
                        ps[:, : HH * DK].rearrange("p (h k) -> p h k", h=HH),
                    )

            def emit_wo(ots, sc, dc, st):
                """Output projection chunk: y[sc-rows st, dc] from 4 pair ots."""
                psy = psB.tile([P, 512], F32, tag="ps")
                for p in range(NPAIR):
                    nc.tensor.matmul(
                        psy[:, :DC],
                        ots[p][:, st * P : (st + 1) * P],
                        wos_all[:, p * D + dc * DC : p * D + (dc + 1) * DC],
                        start=(p == 0), stop=(p == NPAIR - 1),
                    )
                ys = y_p.tile([P, DC], F32, tag="ysb")
                nc.vector.tensor_copy(ys[:], psy[:, :DC])
                nc.sync.dma_start(
                    y[(sc * 4 + st) * P : (sc * 4 + st + 1) * P,
                      dc * DC : (dc + 1) * DC],
                    ys[:],
                )

            bg = []

            def run_unit(u):
                if u[0] == "k":
                    emit_kproj(u[1], u[2], u[3])
                elif u[0] == "v":
                    emit_vproj(u[1], u[2], u[3])
                else:
                    emit_wo(u[1], u[2], u[3], u[4])

            def xdep(u):
                """Highest x-chunk a unit depends on (-1 = none)."""
                if u[0] == "k":
                    return u[2]
                if u[0] == "v":
                    return u[2] // 4
                return -1

            def drain(n, max_x=99):
                """Emit up to n background units whose x-chunk is ready."""
                for _ in range(n):
                    if bg and xdep(bg[0]) <= max_x:
                        run_unit(bg.pop(0))

            def need(u):
                while u in bg:
                    run_unit(bg.pop(0))

            emit_kproj(0, 0, 0)
            emit_kproj(0, 0, 1)

            # ones column (col DK of each per-head block) + remaining weights
            nc.vector.memset(
                v_ones[:].rearrange("p (t h c) -> p (t h) c", h=HL, c=VW)[
                    :, :, DK : DK + 1
                ],
                1.0,
            )
            nc.scalar.dma_start(wv_sb[:], wv[:])
            for p in range(NPAIR):
                nc.sync.dma_start(wqt[p][:], wq[p])

            # consumption order: pair 0's g-loop pulls k(0, chunk) and
            # v(0, tile) incrementally; later pairs' K chunks follow.
            units = []
            for sc in range(1, NSC):  # k(0,sc) just-in-time with v(0) tiles
                units += [("v", 0, tt, ph) for tt in range((sc - 1) * 4, sc * 4)
                          for ph in range(2)]
                units += [("k", 0, sc, ph) for ph in range(2)]
            units += [("v", 0, tt, ph) for tt in range(12, 16) for ph in range(2)]
            units += [("k", 1, sc, ph) for sc in range(NSC) for ph in range(2)]
            for sc in range(NSC):  # k(2,sc) just-in-time with v(1) tiles
                units += [("k", 2, sc, ph) for ph in range(2)]
                units += [("v", 1, tt, ph) for tt in range(sc * 4, (sc + 1) * 4)
                          for ph in range(2)]
            units += [("k", 3, sc, ph) for sc in range(NSC) for ph in range(2)]
            if interleave:
                bg.extend(units)
            else:
                for u in units:
                    run_unit(u)

            nc.scalar.dma_start(wos_all[:], wo[:])

            # --- attention + output projection ---
            for sc in range(NSC):
                ots = []
                for p in range(NPAIR):
                    # Q projection for (p, sc)
                    psq = psB.tile([P, 512], F32, tag="ps")
                    for dt in range(ND):
                        nc.tensor.matmul(
                            psq[:], wqt[p][:, dt * P : (dt + 1) * P],
                            xt[dt][:, sc * 512 : (sc + 1) * 512],
                            start=(dt == 0), stop=(dt == ND - 1),
                        )
                    qtb = qtb_p.tile([P, 512], F32R, tag="qtb")
                    nc.vector.tensor_copy(qtb[:], psq[:])

                    po_e = psB.tile([P, 512], F32, tag="ps")
                    po_o = psB.tile([P, 512], F32, tag="ps")
                    half = 2 * p // HH
                    for g in range(NT):
                        need(("k", p, g // 4, 1))
                        need(("v", half, g, 1))
                        pse = psS.tile([P, 1024], F32, tag="sc")
                        for h in range(2):
                            nc.tensor.matmul(
                                pse[:, h * 512 : (h + 1) * 512],
                                kt[p][h * DK : (h + 1) * DK, g * P : (g + 1) * P],
                                qtb[h * DK : (h + 1) * DK, :],
                                start=True, stop=True,
                            )
                        et = et_p.tile([P, 1024], BF16, tag="exp")
                        nc.scalar.activation(
                            et[:], pse[:], mybir.ActivationFunctionType.Exp,
                            scale=float(scale),
                        )
                        for h, po in ((0, po_e), (1, po_o)):
                            nc.tensor.matmul(
                                po[:VW, :],
                                v_ones[
                                    :,
                                    (g * HL + 2 * p + h) * VW : (g * HL + 2 * p + h + 1) * VW,
                                ],
                                et[:, h * 512 : (h + 1) * 512],
                                start=(g == 0), stop=(g == NT - 1),
                            )
                        # during the first pair, only drain units whose x
                        # chunk has landed (x streams in behind the g-loop)
                        drain(drain_per_g,
                              max_x=(g // 4 + 1) if (sc == 0 and p == 0) else 99)

                    # move accumulators to SBUF (frees PSUM banks), then
                    # normalize: rows 0:64 divided by row 64 (sum of exp)
                    ot = ot_p.tile([P, 512], BF16, tag="ot")
                    ots.append(ot)
                    for h, po in ((0, po_e), (1, po_o)):
                        pos = po_p.tile([VW, 512], F32, tag="po")
                        nc.vector.tensor_copy(pos[:], po[:VW, :])
                        rl = rl_p.tile([VW, 512], F32, tag="rl")
                        nc.vector.reciprocal(rl[DK : DK + 1, :], pos[DK : DK + 1, :])
                        # partition_broadcast reads physical partition 0:
                        # hop the row down first
                        rl0 = rl0_p.tile([1, 512], F32, tag="rl0")
                        nc.sync.dma_start(rl0[:], rl[DK : DK + 1, :])
                        rb = rb_p.tile([DK, 512], F32, tag="rb")
                        nc.gpsimd.partition_broadcast(rb[:], rl0[:], channels=DK)
                        if h == 0:
                            nc.vector.tensor_mul(ot[:DK, :], pos[:DK, :], rb[:])
                        else:
                            tmp = tmp_p.tile([DK, 512], BF16, tag="tmp")
                            nc.vector.tensor_mul(tmp[:], pos[:DK, :], rb[:])
                            nc.sync.dma_start(ot[DK:P, :], tmp[:])

                # output projection for this s-chunk: deferred into the next
                # s-chunk's PE slack so the normalize tail of the last pair
                # never stalls the pipeline
                for dc in range(NDC):
                    for st in range(4):
                        bg.append(("wo", ots, sc, dc, st))
            drain(len(bg))

    nc.compile()
    return nc


_NC_CACHE = {}


def _get_nc(S, D, HL):
    key = (S, D, HL)
    if key not in _NC_CACHE:
        _NC_CACHE[key] = build_nc(S, D, HL)
    return _NC_CACHE[key]


def prep_core_inputs(x_b, wq_l, wk_l, wv_l, wo_l):
    """Per-core input dict from logical per-core slices.

    x_b [S,D]; wq_l/wk_l/wv_l [HL,D,64]; wo_l [HL*64,D]. Weights are
    reshuffled host-side into partition-major layouts (see build_nc).
    """
    S, D = x_b.shape
    HL = wq_l.shape[0]
    ND, NPAIR, NT = D // P, HL // 2, S // P
    bf = ml_dtypes.bfloat16

    def qk_prep(w):
        return np.ascontiguousarray(
            w.reshape(NPAIR, 2, ND, P, DK).transpose(0, 3, 2, 1, 4)
            .reshape(NPAIR, P, ND * P)
        ).astype(bf)

    return {
        "x": np.ascontiguousarray(x_b),
        "wq": qk_prep(wq_l),
        "wk": qk_prep(wk_l),
        "wv": np.ascontiguousarray(
            wv_l.reshape(HL, ND, P, DK).transpose(2, 1, 0, 3)
            .reshape(P, ND * HL * DK)
        ).astype(bf),
        "wo": np.ascontiguousarray(
            wo_l.reshape(NPAIR, P, D).transpose(1, 0, 2).reshape(P, NPAIR * D)
        ).astype(bf),
    }


def make_in_maps(x, W_Q, W_K, W_V, W_O, n_cores=8):
    """Shard full inputs into per-core in_maps (DP over batch x TP over heads)."""
    B = x.shape[0]
    H = W_Q.shape[0]
    n_groups = n_cores // B
    HL = H // n_groups
    in_maps = []
    for c in range(n_cores):
        b, g = c % B, c // B
        hs = slice(g * HL, (g + 1) * HL)
        in_maps.append(prep_core_inputs(
            x[b], W_Q[hs], W_K[hs], W_V[hs],
            W_O[g * HL * DK : (g + 1) * HL * DK],
        ))
    return in_maps


def kernel(x, W_Q, W_K, W_V, W_O):
    from concourse.bass_utils import run_bass_kernel_spmd

    B, S, D = x.shape
    H = W_Q.shape[0]
    n_cores = 8
    HL = H // (n_cores // B)
    nc = _get_nc(S, D, HL)
    in_maps = make_in_maps(x, W_Q, W_K, W_V, W_O, n_cores)
    res = run_bass_kernel_spmd(nc, in_maps, core_ids=list(range(n_cores)))
    y = np.empty((B, S, D), dtype=np.float32)
    for b in range(B):
        y[b] = res.results[b]["y"]
        for g in range(1, n_cores // B):
            y[b] += res.results[g * B + b]["y"]
    return y
